# revision 2
# baseline (speedup 1.0000x reference)
"""ConsensusAttention Trainium2 kernel.

Shapes (hardcoded): levels [B=8, N=1024, L=6, D=128] fp32.
Sharding: batch b across the 8 cores; each core runs all L=6 heads.

Math per (b, l):
  q = x, k = x / ||x||, sim[i, j] = (q_i . k_j) / sqrt(D)
  sim[i, i] = -0.0005 ; sim[i, j] = -inf where grid_dist(i, j) > 2
  out = softmax_j(sim) @ x

Kernel structure:
  * Scores computed transposed, S'[j, i] = x_j . x_i (keys j on
    partitions): per-key 1/(sqrt(D)||x_j||) is the Exp activation's
    per-partition scale, and attn@V contracts over j.
  * UNCLAMPED uniform windows: jb's score window is always tokens
    [128jb-64, 128jb+192) against a zero-padded XT, so every E frame has
    its window at cols [64,320) of a 384-wide slot — no edge cases.
  * Score PSUM banks hold a PAIR of j-blocks [128, 512]; the mask bias
    (-60000 on masked/out-of-range entries, fp16) is ONE 512-col identity
    matmul that also zero-initializes the bank.
  * norm2 = sum_d x^2: DVE square of XT, then eight 1-col PE reduction
    matmuls (lhsT = squared-XT chunk, rhs = ones).  rs = Exp(-.5*Ln(D*n2));
    Ln+Exp both live in the 'natural_log_exp_and_others' ACT table set
    (table map patched so the compiler picks it) -> ONE table load total.
  * V carries a host-prefilled ones-column so attn @ [V|1] yields
    numerator and denominator in one PSUM tile; the self-attention
    diagonal rides as a c0*I matmul into each accumulation.
  * Finalize = a single tensor_scalar divide (numerator/denominator
    straight out of PSUM), alternating DVE/GPSIMD by output block.
  * Software-pipelined flat (head, pair) slot loop: scores lead, attn@V
    lags two pair-slots, next pair's transposes/norms are spread into
    fixed slots of the previous head; fp16 I/O in DMA-friendly
    [L, 128, NB, D(+1)] layouts (2KB descriptors), host packs/unpacks.
"""

from contextlib import ExitStack

import numpy as np

import concourse.bacc as bacc
import concourse.tile as tile
from concourse import mybir
from concourse.bass_utils import run_bass_kernel_spmd

B, N, L, D = 8, 1024, 6, 128
NB = N // 128  # 8 token blocks of 128
NP = NB // 2  # 4 block pairs
GRID = 32
RADIUS = 2.0
SELF_VAL = -0.0005
F32 = mybir.dt.float32
F16 = mybir.dt.float16
F8 = mybir.dt.float8e5
I32 = mybir.dt.int32

XPAD_L, XPAD_R = 64, 192  # window of jb = xt cols [128jb, 128jb+256)
XT_W = XPAD_L + N + XPAD_R


def _patch_act_tables():
    """Make Ln and Exp both resolve to 'natural_log_exp_and_others' so the
    compiler emits a single LoadActFuncSet.  Set ids stay aligned with
    act_info.json (only membership of other sets is filtered), so the
    emitted act_func_set_id is valid on hardware."""
    import functools

    import concourse.hw_specs as hw_specs

    if getattr(hw_specs, "_act_tables_patched", False):
        return
    orig = hw_specs.get_activation_tables

    def patched(arch):
        tabs = dict(orig(arch))
        exp_t = mybir.ActivationFunctionType.Exp
        ln_t = mybir.ActivationFunctionType.Ln
        combined = None
        for name, fns in tabs.items():
            if exp_t in fns and ln_t in fns:
                combined = name
                break
        if combined is not None:
            out = {}
            for name, fns in tabs.items():
                if name != combined:
                    fns = set(fns) - {exp_t, ln_t}
                out[name] = fns
            tabs = out
        return tabs

    patched = functools.cache(patched)
    hw_specs._act_tables_patched = True
    hw_specs.get_activation_tables = patched
    bacc.get_activation_tables = patched
    try:
        import concourse.bass_interp as bass_interp

        bass_interp.get_activation_tables = patched
    except ImportError:
        pass


def _build_constants():
    yy, xx = np.meshgrid(np.arange(GRID), np.arange(GRID), indexing="ij")
    coors = np.stack([yy.ravel(), xx.ravel()], axis=-1).astype(np.float32)
    dist = np.sqrt(((coors[:, None, :] - coors[None, :, :]) ** 2).sum(-1))
    bad = (dist > np.float32(RADIUS)) | np.eye(N, dtype=bool)  # [j, i] masked

    # fp8e5 (e5m2) mask: -57344 is the most negative exactly-representable
    # value and rs*(-57344) <= -600 for any plausible input -> exp == 0.
    # DoubleRow matmuls contract over 2 k-tile planes; plane 1 is zero.
    mbv = np.full((NP, 128, 512), -60000.0, np.float32)
    for jb in range(NB):
        w0 = jb * 128 - 64
        half = (jb % 2) * 256
        lo, hi = max(w0, 0), min(w0 + 256, N)
        mbv[jb // 2, :, half + (lo - w0) : half + (hi - w0)] = np.where(
            bad[jb * 128 : (jb + 1) * 128, lo:hi], -60000.0, 0.0
        )
        assert bad[jb * 128 : (jb + 1) * 128, :lo].all()
        assert bad[jb * 128 : (jb + 1) * 128, hi:].all()
    mb = mbv.astype(np.float16)

    ident = np.eye(128, dtype=np.float16)
    c0i = (np.exp(np.float32(SELF_VAL)) * np.eye(128)).astype(np.float16)
    return mb, np.stack([ident, c0i])


def _emit(tc: tile.TileContext, ctx: ExitStack, xh, mb, cns, out):
    nc = tc.nc
    const = ctx.enter_context(tc.tile_pool(name="const", bufs=1))
    xin = ctx.enter_context(tc.tile_pool(name="xin", bufs=1))
    xtp = ctx.enter_context(tc.tile_pool(name="xtp", bufs=3))
    sqp = ctx.enter_context(tc.tile_pool(name="sqp", bufs=3))
    epool = ctx.enter_context(tc.tile_pool(name="epool", bufs=2))
    stp = ctx.enter_context(tc.tile_pool(name="stp", bufs=2))
    small = ctx.enter_context(tc.tile_pool(name="small", bufs=1))
    tp = ctx.enter_context(tc.tile_pool(name="tp", bufs=1, space="PSUM"))
    sp = ctx.enter_context(tc.tile_pool(name="sp", bufs=3, space="PSUM"))
    op = ctx.enter_context(tc.tile_pool(name="op", bufs=3, space="PSUM"))
    npp = ctx.enter_context(tc.tile_pool(name="npp", bufs=1, space="PSUM"))

    # DRAM layouts are DMA-friendly: xh [L, 128, NB, D+1] (ones prefilled
    # by host), out [L, 128, NB, D]; both give 2KB contiguous elements.
    xh_v = xh.rearrange("l p b d -> p l b d")
    out_v = out.rearrange("l p b d -> p l b d")
    mb_v = mb.rearrange("j p c -> p j c")

    # --- input DMAs on SP's HWDGE queue; head-0 data first.  The identity
    # matrices are generated on-device (memset + affine_select diagonal) so
    # no constant DMA sits ahead of the head-0 chain.
    xh_all = xin.tile([128, L, NB, D + 1], F16, name="xh_all")
    nc.sync.dma_start(out=xh_all[:, 0, 0:4], in_=xh_v[:, 0, 0:4])
    ident = const.tile([128, 128], F16, name="ident_sb")
    nc.sync.dma_start(out=ident, in_=cns[0])
    mb_sb = const.tile([128, NP, 512], F16, name="mb_sb")
    nc.sync.dma_start(out=xh_all[:, 0, 4:8], in_=xh_v[:, 0, 4:8])
    nc.sync.dma_start(out=mb_sb[:, 0:1], in_=mb_v[:, 0:1])
    c0ih = const.tile([128, 128], F16, name="c0ih")
    nc.sync.dma_start(out=c0ih, in_=cns[1])
    nc.sync.dma_start(out=mb_sb[:, 1:], in_=mb_v[:, 1:])
    nc.sync.dma_start(out=xh_all[:, 1], in_=xh_v[:, 1])
    nc.sync.dma_start(out=xh_all[:, 2:], in_=xh_v[:, 2:])
    ones16 = const.tile([128, 1], F16, name="ones16")
    nc.vector.memset(ones16, 1.0)

    n2_ps = npp.tile([128, L, NB], F32, name="n2_ps")
    lnn = small.tile([128, L, NB], F32, name="lnn")
    rs_all = small.tile([128, L, NB], F32, name="rs_all")

    def emit_transpose(l, b0=0, b1=NB, tiles=None):
        """PE transposes + DVE copy + DVE square over blocks [b0, b1).  xt
        cols [64, 1088) are tokens; pads stay zero (memset once per
        rotating buffer)."""
        if tiles is None:
            xt = xtp.tile([128, XT_W], F16, tag="xt", name=f"xt_{l}")
            sq = sqp.tile([128, N], F16, tag="sq", name=f"sq_{l}")
            if l < 3:
                nc.gpsimd.memset(xt[:, 0:XPAD_L], 0.0)
                nc.gpsimd.memset(xt[:, XPAD_L + N :], 0.0)
        else:
            xt, sq = tiles
        if b0 != 0:
            pt = sp.tile([128, (b1 - b0) * 128], F16, tag="s", name=f"pt_{l}_{b0}")
        else:
            pt = tp.tile([128, (b1 - b0) * 128], F16, tag="pt", name=f"pt_{l}_{b0}")
        for b in range(b0, b1):
            nc.tensor.matmul(
                pt[:, (b - b0) * 128 : (b - b0 + 1) * 128],
                lhsT=xh_all[:, l, b, 0:D],
                rhs=ident,
                is_transpose=True,
                start=(b == b0),
                stop=(b == b1 - 1),
            )
        lo, hi = XPAD_L + b0 * 128, XPAD_L + b1 * 128
        nc.vector.tensor_copy(out=xt[:, lo:hi], in_=pt)
        nc.vector.tensor_mul(sq[:, b0 * 128 : b1 * 128], xt[:, lo:hi], xt[:, lo:hi])
        return xt, sq

    def emit_reduces(l, sq, b0=0, b1=NB):
        for b in range(b0, b1):
            nc.tensor.matmul(
                n2_ps[:, l, b : b + 1],
                lhsT=sq[:, b * 128 : (b + 1) * 128],
                rhs=ones16,
                start=True,
                stop=True,
            )

    def emit_rs_act(l, b0=0, b1=NB):
        nc.scalar.activation(
            lnn[:, l, b0:b1],
            n2_ps[:, l, b0:b1],
            mybir.ActivationFunctionType.Ln,
            scale=float(D),
        )
        nc.scalar.activation(
            rs_all[:, l, b0:b1],
            lnn[:, l, b0:b1],
            mybir.ActivationFunctionType.Exp,
            scale=-0.5,
        )

    def emit_rs(l, sq, b0=0, b1=NB):
        emit_reduces(l, sq, b0, b1)
        emit_rs_act(l, b0, b1)

    state = {}  # per-head rotating tiles

    def emit_scores(l, p):
        st = state[l]
        xt = st["xt"]
        s_ps = sp.tile([128, 512], F32, tag="s", name=f"s_{l}_{p}")
        st["s_ps"][p] = s_ps
        # mask bias zero-initializes the pair bank; scores accumulate.
        nc.tensor.matmul(s_ps, lhsT=ident, rhs=mb_sb[:, p, :], start=True, stop=False)
        for h in range(2):
            jb = 2 * p + h
            nc.tensor.matmul(
                s_ps[:, h * 256 : (h + 1) * 256],
                lhsT=xt[:, XPAD_L + jb * 128 : XPAD_L + (jb + 1) * 128],
                rhs=xt[:, jb * 128 : jb * 128 + 256],
                start=False,
                stop=True,
                skip_group_check=True,
            )

    def emit_exps(l, p):
        st = state[l]
        s_ps = st["s_ps"].pop(p)
        for h in range(2):
            jb = 2 * p + h
            nc.scalar.activation(
                st["e"][:, jb, 64:320],
                s_ps[:, h * 256 : (h + 1) * 256],
                mybir.ActivationFunctionType.Exp,
                scale=rs_all[:, l, jb : jb + 1],
            )

    def emit_attnv(l, jb):
        """attn@[V|1] matmuls consuming E slot jb; closes output block
        ib = jb-1 (and ib = jb at the final slot)."""
        st = state[l]
        e_all, o_ps, stage = st["e"], st["o_ps"], st["stage"]
        for ib in (jb - 1, jb, jb + 1):
            if ib < 0 or ib >= NB:
                continue
            first = jb == max(ib - 1, 0)
            last = jb == min(ib + 1, NB - 1)
            if first:
                o_ps[ib] = op.tile([128, D + 1], F32, tag="o", name=f"o_{l}_{ib}")
            nc.tensor.matmul(
                o_ps[ib],
                lhsT=e_all[:, jb, (ib - jb + 1) * 128 : (ib - jb + 2) * 128],
                rhs=xh_all[:, l, jb, :],
                start=first,
                stop=last,
            )
            if first:
                # first != last always (every block has >= 2 contributors)
                nc.tensor.matmul(
                    o_ps[ib],
                    lhsT=c0ih,
                    rhs=xh_all[:, l, ib, :],
                    start=False,
                    stop=False,
                )
            if last:
                ot = o_ps.pop(ib)
                rcp = small.tile([128, 1], F32, tag="rcp", name=f"rcp_{l}_{ib}", bufs=6)
                nc.vector.reciprocal(rcp, ot[:, D : D + 1])
                nc.vector.tensor_scalar_mul(stage[:, ib, :], ot[:, 0:D], rcp)

    def open_head(l, xt):
        e_all = epool.tile([128, NB, 384], F16, tag="e", name=f"e_{l}")
        if l < 2:
            nc.gpsimd.memset(e_all[:, :, 0:64], 0.0)
            nc.gpsimd.memset(e_all[:, :, 320:384], 0.0)
        state[l] = {
            "xt": xt,
            "e": e_all,
            "stage": stp.tile([128, NB, D], F16, tag="stage", name=f"stage_{l}"),
            "o_ps": {},
            "s_ps": {},
        }

    # --- prologue: head 0 in half-block chains (first scores/exps depend
    # only on blocks 0-3), head 1's rs deferred into the slot loop.
    t0 = emit_transpose(0, 0, 4)
    emit_rs(0, t0[1], 0, 4)
    emit_transpose(0, 4, NB, tiles=t0)
    emit_rs(0, t0[1], 4, NB)
    t1 = emit_transpose(1)
    open_head(0, t0[0])
    open_head(1, t1[0])
    pend = {1: t1}  # head -> (xt, sq) staged by slot extras

    # Extras spread into the flat slot stream: for pair k>=1,
    #   slot 8k-4: T(2k);  slot 8k-2: rs(2k) + T(2k+1);  slot 8k: rs(2k+1).
    NSLOT = L * NP
    for s in range(NSLOT + 3):
        if s < NSLOT:
            emit_scores(s // 4, s % 4)
        e = s - 1
        if 0 <= e < NSLOT:
            emit_exps(e // 4, e % 4)
        if s == 0:
            emit_reduces(1, pend[1][1])
        if s == 1:
            emit_rs_act(1)
        if s == NSLOT:
            # tail collapse: the last three attnV pairs run right behind
            # the final exps instead of the steady-state lag.
            for t in (NSLOT - 3, NSLOT - 2, NSLOT - 1):
                l2 = t // 4
                emit_attnv(l2, 2 * (t % 4))
                emit_attnv(l2, 2 * (t % 4) + 1)
                if t % 4 == 2:
                    # ib <= 4 are closed by jb=5; block 5 closes at jb=6
                    nc.sync.dma_start(
                        out=out_v[:, l2, 0:5, :], in_=state[l2]["stage"][:, 0:5, :]
                    )
                if t % 4 == 3:
                    st = state.pop(l2)
                    assert not st["o_ps"]
                    nc.sync.dma_start(
                        out=out_v[:, l2, 5:8, :], in_=st["stage"][:, 5:8, :]
                    )
            break
        t = s - 3
        if t >= 0:
            l2 = t // 4
            emit_attnv(l2, 2 * (t % 4))
            emit_attnv(l2, 2 * (t % 4) + 1)
            if t % 4 == 3:
                st = state.pop(l2)
                assert not st["o_ps"]
                nc.sync.dma_start(out=out_v[:, l2], in_=st["stage"])
        if s % 8 == 2 and s >= 2:  # slot 8k-6: T(2k)
            le = (s + 6) // 4
            if le < L:
                pend[le] = emit_transpose(le)
        if s % 8 == 3 and s >= 3:  # slot 8k-5: reduces(2k)
            le = (s + 5) // 4
            if le < L:
                emit_reduces(le, pend[le][1])
        if s % 8 == 4 and s >= 4:  # slot 8k-4: rs(2k), T(2k+1)
            le = (s + 4) // 4
            if le < L:
                emit_rs_act(le)
                open_head(le, pend[le][0])
                if le + 1 < L:
                    pend[le + 1] = emit_transpose(le + 1)
        if s % 8 == 5 and s >= 5:  # slot 8k-3: reduces(2k+1)
            lo = (s + 3) // 4 + 1
            if lo < L:
                emit_reduces(lo, pend[lo][1])
        if s % 8 == 6 and s >= 6:  # slot 8k-2: rs(2k+1)
            lo = (s + 2) // 4 + 1
            if lo < L:
                emit_rs_act(lo)
                open_head(lo, pend[lo][0])


def build_nc():
    _patch_act_tables()
    nc = bacc.Bacc("TRN2", target_bir_lowering=False, debug=False, num_devices=B)
    xh = nc.dram_tensor("xh", [L, 128, NB, D + 1], F16, kind="ExternalInput").ap()
    mb = nc.dram_tensor("mb", [NP, 128, 512], F16, kind="ExternalInput").ap()
    cns = nc.dram_tensor("cns", [2, 128, 128], F16, kind="ExternalInput").ap()
    out = nc.dram_tensor("out", [L, 128, NB, D], F16, kind="ExternalOutput").ap()
    with tile.TileContext(nc) as tc:
        with ExitStack() as ctx:
            _emit(tc, ctx, xh, mb, cns, out)
    nc.compile()
    return nc


_NC = None


def _get_nc():
    global _NC
    if _NC is None:
        _NC = build_nc()
    return _NC


def run_spmd(levels: np.ndarray, trace: bool = False):
    """Run on the 8 NeuronCores; returns (out [B,N,L,D] fp32, exec_ns|None)."""
    levels = np.ascontiguousarray(levels, dtype=np.float32)
    assert levels.shape == (B, N, L, D), levels.shape
    mb, cns = _build_constants()
    nc = _get_nc()
    # host packs [N, L, D] -> [L, 128, NB, D+1] fp16 with a ones column
    xs = levels.astype(np.float16).reshape(B, NB, 128, L, D)
    xh = np.ones((B, L, 128, NB, D + 1), np.float16)
    xh[..., :D] = xs.transpose(0, 3, 2, 1, 4)
    in_maps = [{"xh": xh[b], "mb": mb, "cns": cns} for b in range(B)]
    res = run_bass_kernel_spmd(nc, in_maps, core_ids=list(range(B)), trace=trace)
    # host unpacks [L, 128, NB, D] -> [N, L, D]
    outs = np.stack([res.results[b]["out"] for b in range(B)])
    out = (
        outs.astype(np.float32)
        .transpose(0, 3, 2, 1, 4)  # b, nb, p, l, d
        .reshape(B, N, L, D)
    )
    return out, res.exec_time_ns


def kernel(levels: np.ndarray) -> np.ndarray:
    out, _ = run_spmd(levels, trace=False)
    return out
